# revision 8
# baseline (speedup 1.0000x reference)
"""2-layer LSTM encoder (batch collapsed into recurrence) on TRN2.

Single-core full-width implementation (the axon/PJRT stack on this pod
cannot execute remote-DMA descriptor instructions, and ncfw collectives
cost ~270us per call here — so no viable per-step cross-core exchange).

  GEMM0: x_pre0 = seq @ W_ih0.T + b0    (on-device, fp32 -> bf16 to HBM)
  rec0 : h0(t) = cell(x_pre0(t) + W_hh0 @ h0(t-1))   1008 steps
  GEMM1: x_pre1 = H0 @ W_ih1.T + b1     (bf16)
  rec1 : h1(t) = cell(x_pre1(t) + W_hh1 @ h1(t-1))

Layout per step: 6144 gate rows as [128 partitions x 48 psum cols]
(cols 0..35 = (i,f,o) x 12 unit-blocks; 36..47 = g x 12). The cell runs
across partitions; h [128,12] bf16 feeds the next matvec directly.
Weights stationary (bf16, fast-weight-load), fp32 PSUM accumulation.

Key perf structure (measured on this axon/PJRT stack):
 - For_i bodies larger than ~1 step (~600 instrs) thrash instruction
   fetch (~1.8ms/step vs ~80us/step), so the recurrence is a per-step
   For_i with a minimal body.
 - Dynamic (register-indexed) ds() offsets live ONLY on DMA operands
   (x_pre prefetch src, H0 archive dst) — never on engine-op access
   patterns, which is where they are catastrophically slow.
 - x_pre(t+1) is prefetched into a fixed SBUF slot at iteration end
   (prologue loads t=0; xp stride is S+1 so the last prefetch stays in
   bounds) and folded into PSUM by PE via an identity-weight matmul
   issued FIRST in the accumulation group (start=True clears
   has_written bank-wide), so the cell reads gates straight from PSUM.
"""

import sys

sys.path.insert(0, "/opt/trn_rl_repo")
import numpy as np
import ml_dtypes
import concourse.bass as bass
import concourse.bacc as bacc
import concourse.mybir as mybir
from concourse import tile
from concourse.tile_rust import add_dep_helper
from contextlib import ExitStack

F32 = mybir.dt.float32
BF16 = mybir.dt.bfloat16
AF = mybir.ActivationFunctionType
PE = mybir.EngineType.PE

B, T, D, H = 16, 64, 256, 1536
NB = H // 128          # 12 unit blocks
NJ = 4 * NB            # 48 psum cols
NK = H // 128          # 12 K-tiles (H contraction)
KD = D // 128          # 2 K-tiles (D contraction)

_IOFF, _FOFF, _GOFF, _OOFF = 0, H, 2 * H, 3 * H


def col_gate(j):
    if j < 36:
        return [_IOFF, _FOFF, _OOFF][j % 3], j // 3
    return _GOFF, j - 36


def gate_rows(j):
    goff, blk = col_gate(j)
    return np.arange(goff + 128 * blk, goff + 128 * blk + 128)


def pack_lhsT(W, nk):
    out = np.zeros((128, nk * NJ * 128), dtype=W.dtype)
    for k in range(nk):
        for j in range(NJ):
            out[:, (k * NJ + j) * 128 : (k * NJ + j + 1) * 128] = W[
                gate_rows(j), 128 * k : 128 * (k + 1)
            ].T
    return out


def pack_bias(b):
    out = np.zeros((1, NJ * 128), dtype=b.dtype)
    for j in range(NJ):
        out[0, 128 * j : 128 * (j + 1)] = b[gate_rows(j)]
    return out


def prep_inputs(batch, W_ih0, W_hh0, b_ih0, b_hh0, W_ih1, W_hh1, b_ih1, b_hh1,
                S=None):
    bf = ml_dtypes.bfloat16
    seq = np.ascontiguousarray(
        batch[:, 1:, :].transpose(1, 0, 2).reshape(-1, D)
    ).astype(np.float32)
    if S is not None:
        seq = seq[:S]
    S = seq.shape[0]
    seqt = np.ascontiguousarray(seq.T)
    b0 = (b_ih0 + b_hh0).astype(np.float32)
    b1 = (b_ih1 + b_hh1).astype(np.float32)
    m = {
        "seqt": np.ascontiguousarray(
            seqt.reshape(KD, 128, S).transpose(1, 0, 2).reshape(128, KD * S)
        ).astype(bf),
        "wih0t": pack_lhsT(W_ih0.astype(bf), KD),
        "wih0b": pack_bias(b0.astype(bf)),
        "whh0t": pack_lhsT(W_hh0.astype(bf), NK),
        "wih1t": pack_lhsT(W_ih1.astype(bf), NK),
        "wih1b": pack_bias(b1.astype(bf)),
        "whh1t": pack_lhsT(W_hh1.astype(bf), NK),
        "ident": np.eye(128, dtype=bf),
    }
    return m, S


def build(S=1008, CHK=42):
    assert S % CHK == 0
    NCHU = S // CHK
    GCH = 504 if S % 504 == 0 else S  # GEMM chunk (psum bank)
    NGC = S // GCH

    nc = bacc.Bacc(
        "TRN2",
        target_bir_lowering=False,
        debug=False,
        detect_race_conditions=False,
        num_devices=1,
    )

    seqt_e = nc.declare_dram_parameter("seqt", [128, KD * S], BF16, isOutput=False)
    wih0t_e = nc.declare_dram_parameter("wih0t", [128, KD * NJ * 128], BF16, isOutput=False)
    wih0b_e = nc.declare_dram_parameter("wih0b", [1, NJ * 128], BF16, isOutput=False)
    whh0t_e = nc.declare_dram_parameter("whh0t", [128, NK * NJ * 128], BF16, isOutput=False)
    wih1t_e = nc.declare_dram_parameter("wih1t", [128, NK * NJ * 128], BF16, isOutput=False)
    wih1b_e = nc.declare_dram_parameter("wih1b", [1, NJ * 128], BF16, isOutput=False)
    whh1t_e = nc.declare_dram_parameter("whh1t", [128, NK * NJ * 128], BF16, isOutput=False)
    ident_e = nc.declare_dram_parameter("ident", [128, 128], BF16, isOutput=False)
    hc_e = nc.declare_dram_parameter("hc", [128, 4 * NB + 1], F32, isOutput=True)

    # x_pre staging in HBM, j-major: [128, NJ * SP] with col j*SP + t
    # (bf16). SP = S+1 pads one column so the recurrence xs prefetch of
    # step t+1 stays in bounds at t = S-1.
    SP = S + 1
    xp0_d = nc.dram_tensor("xp0d", [128, NJ * SP], BF16)
    xp1_d = nc.dram_tensor("xp1d", [128, NJ * SP], BF16)

    with tile.TileContext(nc) as tc, ExitStack() as ctx:
        pool = ctx.enter_context(tc.tile_pool(name="main", bufs=1))
        pp = ctx.enter_context(tc.tile_pool(name="ps", bufs=2, space="PSUM"))

        gpool_cm = tc.tile_pool(name="g0", bufs=1)  # freed after GEMM0
        gpool = gpool_cm.__enter__()
        wih0t = gpool.tile([128, KD * NJ * 128], BF16, tag="wih0t")
        wih0b = pool.tile([1, NJ * 128], BF16, tag="wih0b")
        wih1b = pool.tile([1, NJ * 128], BF16, tag="wih1b")
        seqt = gpool.tile([128, KD * S], BF16, tag="seqt")
        arch = pool.tile([128, NB * S], BF16, tag="arch")     # H0 archive
        ident = pool.tile([128, 128], BF16, tag="ident")
        xsb = pool.tile([128, NJ], BF16, tag="xsb")
        gstg = pool.tile([128, GCH], BF16, tag="gstg")
        ones16 = pool.tile([1, S], BF16, tag="ones16")
        cst = pool.tile([128, NB], F32, tag="cst")
        hfin = pool.tile([128, 4 * NB + 1], F32, tag="hfin")
        hbf = pool.tile([128, NB], BF16, tag="hbf")
        sig = pool.tile([128, 36], F32, tag="sig")
        gt = pool.tile([128, NB], F32, tag="gt")
        th = pool.tile([128, NB], F32, tag="th")
        t1 = pool.tile([128, NB], F32, tag="t1")
        t2 = pool.tile([128, NB], F32, tag="t2")

        nc.sync.dma_start(seqt[:], seqt_e[:])
        nc.sync.dma_start(wih0t[:], wih0t_e[:])
        nc.sync.dma_start(wih0b[:], wih0b_e[:])
        nc.sync.dma_start(wih1b[:], wih1b_e[:])
        nc.sync.dma_start(ident[:], ident_e[:])
        nc.vector.memset(hbf[:], 0.0)
        nc.vector.memset(cst[:], 0.0)
        nc.vector.memset(ones16[:], 1.0)
        nc.scalar.activation(t1[:], cst[:], AF.Sigmoid)
        nc.scalar.activation(t2[:], cst[:], AF.Tanh)

        def gemm(lhsT_sb, bias_sb, rhs_of_k, rhs_ones, xp_dram, nk):
            """xp_dram[:, j*S + t] = sum_k lhsT(k,j).T @ rhs_k + bias_j (bf16)"""
            for ch in range(NGC):
                t0c = ch * GCH
                for j in range(NJ):
                    ps = pp.tile([128, GCH], F32, tag="gps", name="gps")
                    for k in range(nk):
                        nc.tensor.matmul(
                            ps[:],
                            lhsT_sb[:, (k * NJ + j) * 128 : (k * NJ + j + 1) * 128],
                            rhs_of_k(k, t0c, GCH),
                            start=(k == 0), stop=False,
                        )
                    nc.tensor.matmul(
                        ps[:],
                        bias_sb[0:1, 128 * j : 128 * (j + 1)],
                        rhs_ones[0:1, t0c : t0c + GCH],
                        start=False, stop=True,
                    )
                    nc.vector.tensor_copy(gstg[:], ps[:])  # cast to bf16
                    nc.sync.dma_start(
                        xp_dram[:, j * SP + t0c : j * SP + t0c + GCH], gstg[:]
                    )

        # --- GEMM0 ---
        seqt_r = seqt.rearrange("p (k t) -> p k t", k=KD)
        gemm(
            wih0t, wih0b,
            lambda k, t0c, chn: seqt_r[:, k, t0c : t0c + chn],
            ones16, xp0_d, KD,
        )
        gpool_cm.__exit__(None, None, None)
        wpool = ctx.enter_context(tc.tile_pool(name="wp", bufs=1))
        bigw = wpool.tile([128, NK * NJ * 128], BF16, tag="bigw")  # 144KB/part
        scr = pp.tile([128, 1], F32, tag="scr")

        def step_body():
            """One recurrence step; all engine-op addressing is static."""
            ps = pp.tile([128, NJ], F32, tag="mv", name="mv")
            # seed the gates with x_pre on PE: ps = I.T @ xs (start=True
            # clears has_written bank-wide, so this must come FIRST; the
            # weight matmuls then accumulate onto it)
            nc.tensor.matmul(ps[:, 0:NJ], ident[:], xsb[:], start=True, stop=False)
            for j in range(NJ):
                for k in range(NK):
                    nc.tensor.matmul(
                        ps[:, j : j + 1],
                        bigw[:, (k * NJ + j) * 128 : (k * NJ + j + 1) * 128],
                        hbf[:, k : k + 1],
                        start=False, stop=(k == NK - 1),
                    )
            # ~3us of dependency-free PE filler: keeps the PE activity
            # window busy across the cell gap so HAM stays at K=8/8 (a
            # >3.4us idle re-throttles the clock to 1.2GHz). Result is
            # kept live via hfin's 49th column.
            for d in range(20):
                nc.tensor.matmul(
                    scr[:], bigw[:, 128 * d : 128 * (d + 1)], ident[:, 0:1],
                    start=(d == 0), stop=(d == 19),
                )
            # cell, reading gates straight from PSUM
            nc.scalar.activation(sig[:], ps[:, 0:36], AF.Sigmoid)
            nc.scalar.activation(gt[:], ps[:, 36:48], AF.Tanh)
            nc.vector.tensor_mul(t2[:], sig[:, 1:36:3], cst[:])
            nc.vector.tensor_mul(t1[:], sig[:, 0:36:3], gt[:])
            nc.vector.tensor_add(cst[:], t1[:], t2[:])
            nc.scalar.activation(th[:], cst[:], AF.Tanh)
            nc.vector.tensor_mul(hbf[:], sig[:, 2:36:3], th[:])  # h, bf16

        def recurrence(whh_e, xp_dram, archive):
            # Body must stay small (~1 step): For_i bodies that exceed a few
            # IRAM blocks thrash instruction fetch (~1.8ms/step vs ~80us).
            # Dynamic ds() offsets live only on DMA operands, never on
            # engine-op access patterns.
            nc.sync.dma_start(bigw[:], whh_e[:])
            xpd_r = xp_dram.rearrange("p (j t) -> p j t", j=NJ)
            xsb_r = xsb.rearrange("p (j one) -> p j one", j=NJ)
            nc.sync.dma_start(xsb_r[:, :, 0:1], xpd_r[:, :, 0:1])  # prologue
            with tc.For_i(0, S, 1) as t:
                step_body()
                # prefetch next step's x_pre (dynamic src; overlaps the
                # cell and next iteration's weight matmuls)
                nc.sync.dma_start(xsb_r[:, :, 0:1], xpd_r[:, :, bass.ds(t + 1, 1)])
                if archive:
                    nc.sync.dma_start(arch[:, bass.ds(t * NB, NB)], hbf[:])

        recurrence(whh0t_e, xp0_d, archive=True)

        sv0 = nc.vector.tensor_copy(hfin[:, 0:NB], hbf[:])
        sv1 = nc.vector.tensor_copy(hfin[:, NB : 2 * NB], cst[:])
        rst = nc.vector.memset(cst[:], 0.0)
        add_dep_helper(rst.ins, sv1.ins, reason="after save")
        rsh = nc.vector.memset(hbf[:], 0.0)
        add_dep_helper(rsh.ins, sv0.ins, reason="after save")

        # --- GEMM1: x_pre1 = H0 @ W_ih1.T + b1 ---
        nc.sync.dma_start(bigw[:], wih1t_e[:])
        arch_r = arch.rearrange("p (t k) -> p t k", k=NB)
        gemm(
            bigw, wih1b,
            lambda k, t0c, chn: arch_r[:, t0c : t0c + chn, k],
            ones16, xp1_d, NK,
        )

        recurrence(whh1t_e, xp1_d, archive=False)

        nc.vector.tensor_copy(hfin[:, 2 * NB : 3 * NB], hbf[:])
        nc.vector.tensor_copy(hfin[:, 3 * NB : 4 * NB], cst[:])
        nc.vector.tensor_copy(hfin[:, 4 * NB : 4 * NB + 1], scr[:])
        nc.sync.dma_start(hc_e[:], hfin[:])

    return nc


def assemble(results):
    h = np.zeros((2, H), np.float32)
    c = np.zeros((2, H), np.float32)
    hc = np.asarray(results[0]["hc"], dtype=np.float32)
    for blk in range(NB):
        u = 128 * blk
        h[0, u : u + 128] = hc[:, blk]
        c[0, u : u + 128] = hc[:, NB + blk]
        h[1, u : u + 128] = hc[:, 2 * NB + blk]
        c[1, u : u + 128] = hc[:, 3 * NB + blk]
    return h, c


def kernel(**inputs):
    """Full-input entry: build + compile + run on TRN2, return (h, c)."""
    from concourse.bass_utils import run_bass_kernel_spmd

    m, S = prep_inputs(**inputs)
    nc = build(S=S)
    nc.finalize()
    res = run_bass_kernel_spmd(nc, [m], [0])
    h, c = assemble(res.results)
    return h, c
